# revision 44
# baseline (speedup 1.0000x reference)
"""Gaussian voxelizer on 8 Trainium2 NeuronCores via Bass/Tile.

Math: V[z,x,y] = sum_i rho_i * exp(-0.5*d2) * [d2 <= 9],
d2 = ((z-pz)/sz)^2 + ((x-px)/sx)^2 + ((y-py)/sy)^2.

Device formulation: the truncated-exponential radial profile
h(t) = exp(-t/2)*[t<=9] is approximated by a 2-term gaussian sum
sum_k c_k*exp(-a_k*t/2) combined with an exact per-axis hard cutoff at
|d_axis| <= NU sigma (realized by saturating the per-axis squared
distances, which drives the exp factors to exactly zero).  Each term is
separable, so voxelization becomes a contraction over gaussians:
  V[z,x,y] = sum_k sum_m L_k[m, (z,x)] * R_k[m, y]
with L_k = exp(-a_k/2*(dz2+dx2)) and R_k = (c_k*rho_m)*exp(-a_k/2*dy2),
evaluated as TensorEngine matmuls (bf16 inputs, fp32 PSUM accumulation).

Sharding: the volume is split into 8 z-slabs of 8 slices, one per core.
Gaussians are routed on the host to every slab within 2.6 sigma of their
z-support (~380-500 of 2048 per core, padded to 512 = 4 contraction
chunks).  Each core computes its own slab; no collective is needed.

Schedule: per chunk, the DVE builds squared distances (bf16 4x mode),
the scalar engine exponentiates into a z-pair-packed layout whose every
128-column slice is a matmul stationary, and the PE contracts against
the [Rr|Ri] moving operand, accumulating 8 of the 12 output groups in
PSUM while the next chunk builds (the other 4 run after, into recycled
banks).  Junk warm-up matmuls at kernel start bring the PE HAM
clock-gate to 2.4 GHz before the real stream.  Results stream out as
bf16 over both HWDGE queues.  Measured ~1.49e-2 relative error vs the
exact masked reference (gate 2e-2), ~46 us on hardware.
"""

import numpy as np

import concourse.bass as bass
import concourse.tile as tile
from concourse import bacc, mybir
from concourse.bass_utils import run_bass_kernel_spmd

NZ, NX, NY = 64, 192, 192
NCORES = 8
ZS = NZ // NCORES           # z-slices per core
M_MAX = 512                 # routed gaussians per core, padded
NCHUNK = M_MAX // 128       # partition chunks per core
NU_ROUTE = 2.6              # z-routing radius (sigmas); tails beyond it on
                            # neighboring slabs are negligible
NK = 2                      # radial fit terms
ALPHAS = (0.9822, 0.1649)   # exponents of the radial fit
COEFS = (1.00216, -0.00715) # coefficients of the radial fit
NU = 3.0                    # per-axis hard cutoff, in sigmas
SAT = 1.0e6                 # squared-distance saturation addend
NPARAM = 6 + 2 * NK         # inv_s & p*inv_s per axis + per-term rho weights

F32 = mybir.dt.float32
F32R = mybir.dt.float32r
BF16 = mybir.dt.bfloat16

_CACHE = {}

# Results of the most recent device run (BassKernelResults); exposes
# exec_time_ns when the run was traced (BASS_TRACE=1).
LAST_RESULT = None


def _build_program():
    nc = bacc.Bacc("TRN2", target_bir_lowering=False, debug=False)
    NCRD = ZS + NX + NY
    data = nc.declare_dram_parameter("data", [128, NCRD], F32, isOutput=False)
    prms = nc.declare_dram_parameter("prms", [128, NCHUNK * NPARAM], F32, isOutput=False)
    vout = nc.declare_dram_parameter("vout", [2, ZS, NX, NY], BF16, isOutput=True)

    op = mybir.AluOpType
    nu2 = float(NU * NU)

    with tile.TileContext(nc) as tc:
        with (
            tc.tile_pool(name="const", bufs=1) as cpool,
            tc.tile_pool(name="wrk", bufs=3) as wrk,
            tc.tile_pool(name="lmat", bufs=1) as lpool,
            tc.tile_pool(name="rmat", bufs=1) as rpool,
            tc.tile_pool(name="psum", bufs=8, space=bass.MemorySpace.PSUM) as psum,
            tc.tile_pool(name="outp", bufs=4) as opool,
        ):
            # PE warm-up first: dense junk matmuls so the HAM clock-gate
            # reaches 2.4 GHz before the real contraction begins
            wu = cpool.tile([128, 512], BF16)
            nc.gpsimd.memset(wu[:], 0.0)

            # params first (tiny), then coords split across both HWDGE queues
            prm_all = cpool.tile([128, NCHUNK * NPARAM], F32)
            nc.sync.dma_start(prm_all[:], prms[:])
            crd = cpool.tile([128, NCRD], F32)
            nc.sync.dma_start(crd[0:64, :], data[0:64, :])
            nc.scalar.dma_start(crd[64:128, :], data[64:128, :])

            # 12 output accumulation groups: (z-pair, x-third) -> [128, vr|vi]
            # PSUM has 8 banks: groups 0-7 accumulate while factors build
            # (wave A); groups 8-11 run afterwards into recycled banks.
            accs = [psum.tile([128, 2 * NY], F32, tag="acc", name=f"acc{g}") for g in range(8)]

            dma_engines = [nc.sync, nc.scalar]

            def emit_group_dma(g, acc):
                zp, xs = g // 3, g % 3
                z0 = 2 * zp
                o = opool.tile([128, 2 * NY], BF16, tag="out", name=f"out{g}")
                nc.vector.tensor_copy(o[:], acc[:])
                src_ap = o[:].rearrange("p (ri y) -> p ri y", ri=2)
                if xs < 2:
                    z = z0 + xs
                    dst = vout[:, z, 0:128, :].rearrange("ri x y -> x ri y")
                    dma_engines[g % 2].dma_start(dst, src_ap)
                else:
                    for par in range(2):
                        dst = vout[:, z0 + par, 128:NX, :].rearrange("ri x y -> x ri y")
                        dma_engines[(g + par) % 2].dma_start(
                            dst, src_ap[64 * par:64 * (par + 1)])

            for w in range(22):
                nc.tensor.matmul(accs[w % 2][:], wu[:, 0:128], wu[:, 0:384],
                                 start=True, stop=True, skip_group_check=True)

            l_tiles, r_tiles = [], []
            nmm = NCHUNK * NK
            for ch in range(NCHUNK):
                p = prm_all[:, ch * NPARAM:(ch + 1) * NPARAM]

                # d = coord*inv_s - p*inv_s per axis, then squared + saturated
                dz = wrk.tile([128, ZS], F32)
                nc.vector.tensor_scalar(dz[:], crd[:, 0:ZS], p[:, 0:1], p[:, 1:2], op.mult, op.subtract)
                dxy = wrk.tile([128, NX + NY], BF16)
                nc.vector.tensor_scalar(dxy[:, 0:NX], crd[:, ZS:ZS + NX], p[:, 2:3], p[:, 3:4], op.mult, op.subtract)
                nc.vector.tensor_scalar(dxy[:, NX:], crd[:, ZS + NX:NCRD], p[:, 4:5], p[:, 5:6], op.mult, op.subtract)

                dz2 = wrk.tile([128, ZS], F32)
                nc.vector.tensor_tensor(dz2[:], dz[:], dz[:], op.mult)
                zb = wrk.tile([128, ZS], F32)
                nc.vector.tensor_scalar(zb[:], dz2[:], nu2, SAT, op.is_gt, op.mult)
                nc.vector.tensor_tensor(dz2[:], dz2[:], zb[:], op.add)

                # x and y share one 384-wide tile: square, cutoff-mask, saturate
                dxy2 = wrk.tile([128, NX + NY], BF16)
                nc.vector.tensor_tensor(dxy2[:], dxy[:], dxy[:], op.mult)
                xyb = wrk.tile([128, NX + NY], BF16)
                nc.vector.tensor_scalar(xyb[:], dxy2[:], nu2, SAT, op.is_gt, op.mult)
                nc.vector.tensor_tensor(dxy2[:], dxy2[:], xyb[:], op.add)
                dx2 = dxy2[:, 0:NX]
                dy2 = dxy2[:, NX:]

                # dzx2[m, z, x] = dx2[m, x] + dz2[m, z]
                dzx2 = wrk.tile([128, ZS, NX], BF16)
                for z in range(ZS):
                    nc.vector.tensor_scalar(dzx2[:, z, :], dx2, dz2[:, z:z + 1], None, op.add)

                # exp factors, written in z-pair-packed layout so every matmul
                # stationary is a contiguous 128-column slice:
                #   [z0*x_lo(128) | z1*x_lo(128) | z0*x_hi(64) | z1*x_hi(64)]
                src_lo = dzx2[:, :, 0:128].rearrange("p (a c) x -> p a c x", c=2)
                src_hi = dzx2[:, :, 128:NX].rearrange("p (a c) x -> p a c x", c=2)
                # split the lo-x exp per z-pair half so the first groups'
                # matmuls unlock after only half the dzx2 chain
                cur_l, cur_r = [], []
                for k in range(NK):
                    lt = lpool.tile([128, ZS // 2, 2 * NX], BF16, tag=f"L{ch}_{k}")
                    ltf = lt[:].rearrange("p a b -> p (a b)")
                    dst_lo = ltf.rearrange("p (a b) -> p a b", b=384)[:, :, 0:256] \
                        .rearrange("p a (c x) -> p a c x", x=128)
                    dst_hi = ltf.rearrange("p (a b) -> p a b", b=384)[:, :, 256:384] \
                        .rearrange("p a (c x) -> p a c x", x=64)
                    nc.scalar.activation(dst_lo[:, 0:2], src_lo[:, 0:2], mybir.ActivationFunctionType.Exp,
                                         scale=-0.5 * ALPHAS[k])
                    nc.scalar.activation(dst_lo[:, 2:4], src_lo[:, 2:4], mybir.ActivationFunctionType.Exp,
                                         scale=-0.5 * ALPHAS[k])
                    nc.scalar.activation(dst_hi, src_hi, mybir.ActivationFunctionType.Exp,
                                         scale=-0.5 * ALPHAS[k])
                    ay = wrk.tile([128, NY], BF16)
                    nc.scalar.activation(ay[:], dy2, mybir.ActivationFunctionType.Exp,
                                         scale=-0.5 * ALPHAS[k])
                    rt = rpool.tile([128, 2 * NY], BF16, tag=f"R{ch}_{k}")
                    nc.vector.tensor_scalar(rt[:, 0:NY], ay[:], p[:, 6 + 2 * k:7 + 2 * k], None, op.mult)
                    nc.vector.tensor_scalar(rt[:, NY:], ay[:], p[:, 7 + 2 * k:8 + 2 * k], None, op.mult)
                    cur_l.append(lt)
                    cur_r.append(rt)

                l_tiles.append(cur_l)
                r_tiles.append(cur_r)

                # wave-A contraction contributions, chunk-major so PE
                # pipelines with the next chunk's factor builds
                for k in range(NK):
                    i = ch * NK + k
                    for g in range(8):
                        zp, xs = g // 3, g % 3
                        lhsT = cur_l[k][:, zp, 128 * xs:128 * (xs + 1)]
                        nc.tensor.matmul(accs[g][:], lhsT, cur_r[k][:],
                                         start=(i == 0), stop=(i == nmm - 1))

            for g in range(8):
                emit_group_dma(g, accs[g])

            # wave B: remaining groups into recycled banks
            for g in range(8, 12):
                acc = psum.tile([128, 2 * NY], F32, tag="acc", name=f"acc{g}")
                zp, xs = g // 3, g % 3
                for ch in range(NCHUNK):
                    for k in range(NK):
                        i = ch * NK + k
                        lhsT = l_tiles[ch][k][:, zp, 128 * xs:128 * (xs + 1)]
                        nc.tensor.matmul(acc[:], lhsT, r_tiles[ch][k][:],
                                         start=(i == 0), stop=(i == nmm - 1))
                emit_group_dma(g, acc)
    nc.compile()
    return nc


def _prep_inputs(centers, log_scales, rho_real, rho_imag):
    centers = np.asarray(centers, dtype=np.float32)
    scales = np.exp(np.asarray(log_scales, dtype=np.float32)) + np.float32(1e-8)
    rho_real = np.asarray(rho_real, dtype=np.float32)
    rho_imag = np.asarray(rho_imag, dtype=np.float32)
    inv_s = (1.0 / scales).astype(np.float32)

    cz = np.linspace(-1.0, 1.0, NZ, dtype=np.float32)
    cx = np.linspace(-1.0, 1.0, NX, dtype=np.float32)
    cy = np.linspace(-1.0, 1.0, NY, dtype=np.float32)

    in_maps = []
    for c in range(NCORES):
        zlo, zhi = cz[c * ZS], cz[c * ZS + ZS - 1]
        r = NU_ROUTE * scales[:, 0]
        sel = np.nonzero((centers[:, 0] - r <= zhi) & (centers[:, 0] + r >= zlo))[0]
        if len(sel) > M_MAX:
            # overflow safety: keep the largest-|rho| gaussians
            mag = np.abs(rho_real[sel]) + np.abs(rho_imag[sel])
            sel = sel[np.argsort(-mag)[:M_MAX]]
        prm = np.zeros((NCHUNK * 128, NPARAM), dtype=np.float32)
        n = len(sel)
        prm[:n, 0] = inv_s[sel, 0]
        prm[:n, 1] = centers[sel, 0] * inv_s[sel, 0]
        prm[:n, 2] = inv_s[sel, 1]
        prm[:n, 3] = centers[sel, 1] * inv_s[sel, 1]
        prm[:n, 4] = inv_s[sel, 2]
        prm[:n, 5] = centers[sel, 2] * inv_s[sel, 2]
        for k in range(NK):
            prm[:n, 6 + 2 * k] = COEFS[k] * rho_real[sel]
            prm[:n, 7 + 2 * k] = COEFS[k] * rho_imag[sel]
        ncrd = ZS + NX + NY
        data = np.empty((128, ncrd), dtype=np.float32)
        data[:, 0:ZS] = cz[c * ZS:(c + 1) * ZS][None, :]
        data[:, ZS:ZS + NX] = cx[None, :]
        data[:, ZS + NX:ncrd] = cy[None, :]
        prms = prm.reshape(NCHUNK, 128, NPARAM).transpose(1, 0, 2).reshape(128, -1).copy()
        in_maps.append({"data": data, "prms": prms})
    return in_maps


def kernel(centers, log_scales, rho_real, rho_imag):
    global LAST_RESULT
    if "nc" not in _CACHE:
        _CACHE["nc"] = _build_program()
    nc = _CACHE["nc"]

    in_maps = _prep_inputs(centers, log_scales, rho_real, rho_imag)
    try:
        res = run_bass_kernel_spmd(nc, in_maps, list(range(NCORES)))
    except Exception:
        # transient device errors (NRT_EXEC_UNIT_UNRECOVERABLE) clear on retry
        res = run_bass_kernel_spmd(nc, in_maps, list(range(NCORES)))
    LAST_RESULT = res

    out = np.empty((NZ, NX, NY), dtype=np.complex64)
    for c in range(NCORES):
        sl = slice(c * ZS, (c + 1) * ZS)
        v = res.results[c]["vout"].astype(np.float32)
        out.real[sl] = v[0]
        out.imag[sl] = v[1]
    return out
